# revision 34
# baseline (speedup 1.0000x reference)
"""LlamaPEER MoE-routing kernel for 8 NeuronCores (TRN2, Bass/Tile).

Data-parallel over B*T (2048 tokens -> 256/core). Expert tables are
host-packed into ONE bf16 [E, 2D] cat table (down|up concatenated; rel err
2.2e-3 vs the 2e-2 gate, fp8 measured 5-7e-2 and fails), so each (token,
slot) needs a single 8KB-descriptor indirect gather: half the HBM bytes
and half the GpSimd descriptor-generation work of the fp32 two-table
scheme.  The kernel is DMA-bound: 64MB of gathers/core at the ~420 GB/s
16-engine ceiling, so everything else hides behind the gather stream.
Per core:
  1. qT = Wq^T @ x^T on PE (fp32 exactly - bf16 routing flips 4.8% of the
     top-k indices and costs 1e-1 rel err).  Chunk 0 computes only the
     first 128 tokens to shorten the head; copies drain on DVE.
  2. Per (half, head) chain: sims on PE, top-8 via DVE max/max_index,
     K x K cross combine + second top-8 + iota-mask index extraction
     (the fused DVE reduce ops don't lower in this walrus build, and all
     DVE reduces run ~2.4us vs Scalar ACT-accum's 1.1us - engine choice
     here is dictated by those measured costs).
  3. Per slot: gather cat row (bf16); down-dot = DVE bf16 product +
     DVE fold to 1024 + Scalar ACT accum_out (elem-paced, so the fold
     halves its cost); silu per-4 on Scalar; diag = (ident*silu)*relu(fs)
     in one DVE two-scalar tensor_scalar; up-proj via PE bf16 diag
     matmuls accumulated in PSUM; copy-out + store per half.
PSUM is split into two 4-bank pools so half-1's accumulator never
WAR-waits on half-0's copy-out; half-1 routes are emitted early so no
route's serial DVE chain lands between late consume chains; gather
buffers (13 x 8KB/partition) self-pace the stream against consume
releases.  Loads spread their ~0.7us issue cost across Sync and Scalar
queues to dodge per-queue DMA-semaphore serialization at startup.
"""

import numpy as np
import ml_dtypes

import concourse.bass as bass
import concourse.tile as tile
from concourse import mybir
from concourse.bass_utils import run_bass_kernel_spmd
from concourse.vector_clock import ScopedClock

N_CORES = 8
B, T, D = 2, 1024, 2048
H, K, DK = 4, 8, 64
E = 16384
NK = 128
TOK = (B * T) // N_CORES  # 256 tokens per core
NSLOT = H * K  # 32 slots per 128-token half
CAT = 2 * D  # concatenated row length
FP = mybir.dt.float32
BF = mybir.dt.bfloat16
I32 = mybir.dt.int32
U32 = mybir.dt.uint32

# --- workaround: this walrus build allows only 1 sync-wait command on the
# final SP drain; split the tile-context drain into 1-wait drains.
_MAX_DRAIN_WAITS = 1


def _patched_drain_and_barrier(self, tick_clock, wait_clock):
    nc = self.nc
    drain_inst = nc.sync.drain()
    wait_clock.add_sem_waits(
        drain_inst.ins, ScopedClock({None: tick_clock.global_clock})
    )
    si = drain_inst.ins.sync_info
    if si is not None and len(si.on_wait) > _MAX_DRAIN_WAITS:
        waits = list(si.on_wait)
        upds = list(si.on_update)
        drain_inst.ins.sync_info = mybir.SyncInfo(
            on_wait=waits[:_MAX_DRAIN_WAITS], on_update=[]
        )
        rest = waits[_MAX_DRAIN_WAITS:]
        while rest:
            extra = nc.sync.drain()
            extra.ins.sync_info = mybir.SyncInfo(
                on_wait=rest[:_MAX_DRAIN_WAITS],
                on_update=upds if len(rest) <= _MAX_DRAIN_WAITS else [],
            )
            rest = rest[_MAX_DRAIN_WAITS:]
    nc.all_engine_barrier()
    popped = nc._tile_sem_poison_stack.pop()
    assert popped is self._sem_poison
    all_sems = list(self.sems.allocated().values())
    for i in range(0, len(all_sems), 8):
        nc.clear_and_free_semaphores(all_sems[i : i + 8])
    nc.all_engine_barrier()


tile.TileContext._drain_and_barrier = _patched_drain_and_barrier

_orig_lower_ordered = tile.TileContext._lower_ordered_insts


def _patched_lower_ordered(self, postordered_blocks):
    # this walrus build supports only one sync-wait command per instruction:
    # hoist extra waits onto same-engine NoOps placed just before.
    for bb_name, insts in postordered_blocks.items():
        new = []
        for inst in insts:
            si = getattr(inst, "sync_info", None)
            eng = getattr(inst, "engine", None)
            if si is not None and eng is not None and len(si.on_wait) > 1:
                waits = list(si.on_wait)
                for w in waits[:-1]:
                    nop = mybir.InstNoOp(
                        name=self.nc.get_next_instruction_name(),
                        sync_info=mybir.SyncInfo(on_wait=[w], on_update=[]),
                        bass_nofuse=True,
                        engine=eng,
                    )
                    new.append(nop)
                inst.sync_info = mybir.SyncInfo(
                    on_wait=[waits[-1]], on_update=list(si.on_update)
                )
            new.append(inst)
        insts[:] = new
    return _orig_lower_ordered(self, postordered_blocks)


tile.TileContext._lower_ordered_insts = _patched_lower_ordered


def _re(ap, dims):
    """Return ap with its free-axis access pattern replaced by `dims`
    (list of [step, count]); keeps the partition dim."""
    return ap.__replace__(ap=[list(ap.ap)[0]] + [list(d) for d in dims])


def build_program():
    nc = bass.Bass("TRN2", target_bir_lowering=False, debug=False)

    NDCH = D // 128  # 16 d-chunks

    # xts/wqs are host-prelayouted to the exact SBUF image (contiguous
    # 16KB-per-partition loads): xts[p, c*TOK+t] = x[t, c*128+p];
    # wqs[p, m*16*128 + c*128 + o] = Wq[c*128+p, m*128+o] (m-major, so the
    # 1MB slice feeding qT chunk m=0 loads first and routing starts early).
    xts_d = nc.dram_tensor("xts", [128, NDCH * TOK], FP, kind="ExternalInput")
    xb_d = nc.dram_tensor("xb", [TOK, D], BF, kind="ExternalInput")
    wqs_d = nc.dram_tensor("wqs", [128, NDCH * 512], FP, kind="ExternalInput")
    kt_d = nc.dram_tensor("keyst", [2 * DK, H * NK], FP, kind="ExternalInput")
    ec_d = nc.dram_tensor("ecat", [E, CAT], BF, kind="ExternalInput")
    id_d = nc.dram_tensor("identb", [128, 128], BF, kind="ExternalInput")
    io_d = nc.dram_tensor("iota64", [128, 64], FP, kind="ExternalInput")
    out_d = nc.dram_tensor("out", [TOK, D], BF, kind="ExternalOutput")

    with tile.TileContext(nc) as tc:
        with (
            tc.tile_pool(name="const", bufs=1) as cpool,
            tc.tile_pool(name="mats", bufs=1) as mpool,
            tc.tile_pool(name="route", bufs=3) as rpool,
            tc.tile_pool(name="persist", bufs=1) as ppool,
            tc.tile_pool(name="gc", bufs=13) as gcpool,
            tc.tile_pool(name="scr", bufs=2) as spool,
            tc.tile_pool(name="dg", bufs=4) as dgpool,
            tc.tile_pool(name="ob", bufs=2) as opool,
            # PSUM split into two 4-bank halves: psa holds half-0's up-proj
            # accumulator; psb time-shares warm/qT/sims (all done early) and
            # then half-1's accumulator, so the half-1 matmuls never WAR-wait
            # on half-0's copy-out (that wait stalled the gather stream ~9us
            # at the half boundary when both accs shared banks).
            tc.tile_pool(name="psa", bufs=1, space="PSUM") as psa,
            tc.tile_pool(name="psb", bufs=1, space="PSUM") as psb,
        ):
            wq_sb = mpool.tile([128, NDCH * 512], FP)
            MW = NDCH * 128  # 2048 columns per m-chunk of wq (m-major)
            # each dma_start costs ~0.7us of serial issue time on its engine
            # and each engine has a small DMA-semaphore pool, so spread the
            # issue load: tiny consts go out on Scalar's queue while Sync
            # issues the MB-sized wq/xt/x loads (5 DMAs, under the pool
            # limit that serialized the xt quarters when Sync had 10).
            ident = cpool.tile([128, 128], BF)
            nc.scalar.dma_start(ident[:], id_d.ap())
            iota = cpool.tile([128, 64], FP)
            nc.scalar.dma_start(iota[:], io_d.ap())
            kt_sb = cpool.tile([2 * DK, H * NK], FP)
            nc.scalar.dma_start(kt_sb[:], kt_d.ap())
            nc.sync.dma_start(wq_sb[:, 0:MW], wqs_d.ap()[:, 0:MW])
            xt_sb = mpool.tile([128, NDCH * TOK], FP)
            # halved so qT chunk matmuls overlap the load stream
            for q2 in range(2):
                nc.sync.dma_start(
                    xt_sb[:, q2 * 8 * TOK : (q2 + 1) * 8 * TOK],
                    xts_d.ap()[:, q2 * 8 * TOK : (q2 + 1) * 8 * TOK],
                )

            # PE warmup: back-to-back matmuls on ident (32KB, lands in ~1us)
            # release the HAM clock throttle while the big loads stream in,
            # so the latency-critical qT chain runs at full clock.
            wps = psb.tile([128, 128], FP, tag="psb", name="warm")
            for w in range(12):
                nc.tensor.matmul(
                    wps[:, 0:128], lhsT=ident[:], rhs=ident[:],
                    start=(w == 0), stop=(w == 11),
                )
            wsb = rpool.tile([128, 128], FP, tag="sim", name="warmout")
            nc.scalar.activation(wsb[:], wps[:, 0:128], mybir.ActivationFunctionType.Copy)

            # trailing loads (consumed from the first consume chain on)
            x_sb = []
            for hf in range(2):
                xh = ppool.tile([128, D], BF, tag=f"x{hf}", name=f"x{hf}")
                nc.sync.dma_start(xh[:], xb_d.ap()[hf * 128 : hf * 128 + 128, :])
                x_sb.append(xh)

            def emit_trailing_wq():
                # issued from the Scalar engine after the first routing chain
                # is emitted, so these 3MB enqueue on the SDMA queues around
                # the first gathers and fill the load->route latency gap.
                for m in range(1, 4):
                    nc.scalar.dma_start(
                        wq_sb[:, m * MW : (m + 1) * MW],
                        wqs_d.ap()[:, m * MW : (m + 1) * MW],
                    )

            qt_sb = ppool.tile([128, 4 * TOK], FP)
            fi_all = [
                ppool.tile([128, NSLOT], I32, tag=f"fi{hf}", name=f"fi{hf}")
                for hf in range(2)
            ]
            fsr_all = [
                ppool.tile([128, NSLOT], FP, tag=f"fsr{hf}", name=f"fsr{hf}")
                for hf in range(2)
            ]
            hid_all = [
                ppool.tile([128, NSLOT], FP, tag=f"hid{hf}", name=f"hid{hf}")
                for hf in range(2)
            ]
            hs2_all = [
                ppool.tile([128, NSLOT], FP, tag=f"hs2{hf}", name=f"hs2{hf}")
                for hf in range(2)
            ]
            acc = {}
            gts = {}

            def emit_qt(m, half=None):
                # qT chunk m: psum_q[p, t] = q[t, m*128+p].  half=0/1 limits
                # to 128 tokens, halving the serial fp32 matmul block on the
                # head critical path (chain 0 needs only half 0's columns).
                if half is None:
                    t0, tw = 0, TOK
                else:
                    t0, tw = half * 128, 128
                pq = psb.tile([128, tw], FP, tag="psb", name=f"pq{m}h{half}")
                for c in range(NDCH):
                    nc.tensor.matmul(
                        pq[:],
                        lhsT=wq_sb[:, m * MW + c * 128 : m * MW + (c + 1) * 128],
                        rhs=xt_sb[:, c * TOK + t0 : c * TOK + t0 + tw],
                        start=(c == 0),
                        stop=(c == NDCH - 1),
                    )
                # copy-out on DVE: Scalar is the consume pacer, DVE's route
                # chain follows this copy in program order anyway.
                nc.vector.tensor_copy(
                    qt_sb[:, m * TOK + t0 : m * TOK + t0 + tw], pq[:]
                )

            def emit_route(hf, h):
                t0 = hf * 128
                ss = []
                ii = []
                for sub in range(2):
                    ps = psb.tile([128, NK], FP, tag="psb", name="ps")
                    nc.tensor.matmul(
                        ps[:],
                        lhsT=qt_sb[
                            sub * 64 : (sub + 1) * 64,
                            h * TOK + t0 : h * TOK + t0 + 128,
                        ],
                        rhs=kt_sb[sub * 64 : (sub + 1) * 64, h * NK : (h + 1) * NK],
                        start=True,
                        stop=True,
                    )
                    sim = rpool.tile([128, NK], FP, tag="sim")
                    nc.scalar.activation(
                        sim[:], ps[:], mybir.ActivationFunctionType.Copy
                    )
                    s = rpool.tile([128, 8], FP, tag="s")
                    nc.vector.max(s[:], sim[:])
                    idx = rpool.tile([128, 8], U32, tag="idx")
                    nc.vector.max_index(idx[:], s[:], sim[:])
                    idf = rpool.tile([128, 8], FP, tag="idf")
                    nc.vector.tensor_copy(idf[:], idx[:])
                    ss.append(s)
                    ii.append(idf)
                # cross combine: [128, 8(k1), 8(k2)]
                alls = rpool.tile([128, 64], FP, tag="alls")
                a3 = _re(alls[:], [[8, 8], [1, 8]])
                nc.vector.tensor_tensor(
                    out=a3,
                    in0=_re(ss[0][:], [[1, 8], [0, 8]]),
                    in1=_re(ss[1][:], [[0, 8], [1, 8]]),
                    op=mybir.AluOpType.add,
                )
                alli = rpool.tile([128, 64], FP, tag="alli")
                ai3 = _re(alli[:], [[8, 8], [1, 8]])
                nc.vector.tensor_scalar(
                    out=ai3,
                    in0=_re(ii[0][:], [[1, 8], [0, 8]]),
                    scalar1=float(NK),
                    scalar2=None,
                    op0=mybir.AluOpType.mult,
                )
                nc.vector.tensor_tensor(
                    out=ai3,
                    in0=ai3,
                    in1=_re(ii[1][:], [[0, 8], [1, 8]]),
                    op=mybir.AluOpType.add,
                )
                fs = rpool.tile([128, 8], FP, tag="fs")
                nc.vector.max(fs[:], alls[:])
                pk = rpool.tile([128, 8], U32, tag="pk")
                nc.vector.max_index(pk[:], fs[:], alls[:])
                pkf = rpool.tile([128, 8], FP, tag="pkf")
                nc.vector.tensor_copy(pkf[:], pk[:])
                # scores: relu on scalar engine
                nc.scalar.activation(
                    fsr_all[hf][:, h * 8 : (h + 1) * 8],
                    fs[:],
                    mybir.ActivationFunctionType.Relu,
                )
                # mask[p, j, n] = (pk[p,j] == iota[p,n]) * alli[p,n]; reduce.
                # Runs on GpSimd: these 512-wide ops were the priciest part
                # of the route chain on DVE, where they stalled the consume
                # mults that pace gather-buffer releases.  GpSimd is safe:
                # the mask only waits on DVE routing outputs, never on
                # gather buffers, so it cannot deadlock the indirect queue,
                # and the gathers that need fi follow it in program order.
                mask = rpool.tile([128, 512], FP, tag="mask", bufs=2)
                m3 = _re(mask[:], [[64, 8], [1, 64]])
                nc.vector.tensor_tensor(
                    out=m3,
                    in0=_re(pkf[:], [[1, 8], [0, 64]]),
                    in1=_re(iota[:], [[0, 8], [1, 64]]),
                    op=mybir.AluOpType.is_equal,
                )
                nc.vector.tensor_tensor(
                    out=m3,
                    in0=m3,
                    in1=_re(alli[:], [[0, 8], [1, 64]]),
                    op=mybir.AluOpType.mult,
                )
                # free-axis reduce isn't available on GpSimd; DVE does it,
                # then GpSimd converts to i32 right before its own gathers.
                fif = rpool.tile([128, 8], FP, tag="fif")
                nc.vector.tensor_reduce(
                    fif[:],
                    m3,
                    axis=mybir.AxisListType.X,
                    op=mybir.AluOpType.add,
                )
                nc.gpsimd.tensor_copy(fi_all[hf][:, h * 8 : (h + 1) * 8], fif[:])

            def emit_gather(hf, h):
                # one cat-row gather per slot: [down | up] in a single 8KB
                # descriptor per token.
                for j in range(K):
                    k = h * 8 + j
                    gc = gcpool.tile([128, CAT], BF, tag="gc")
                    nc.gpsimd.indirect_dma_start(
                        out=gc[:],
                        out_offset=None,
                        in_=ec_d.ap(),
                        in_offset=bass.IndirectOffsetOnAxis(
                            ap=fi_all[hf][:, k : k + 1], axis=0
                        ),
                    )
                    gts.setdefault((hf, h), []).append(gc)

            def emit_consume(hf, h):
                t0 = hf * 128
                if h == 0:
                    if hf == 0:
                        acc[hf] = psa.tile([128, D], FP, tag="acc", name="acc0")
                    else:
                        acc[hf] = psb.tile([128, D], FP, tag="psb", name="acc1")
                # sub-groups of 2 slots: the fused DVE dot (product stream +
                # per-partition accumulate) per slot, then a small silu/hs2
                # batch and the dg+matmuls, so gather buffers release
                # continuously.  The very last chain ends in two 1-slot
                # groups to minimize the tail.
                last = hf == 1 and h == H - 1
                # per-2 silu batches: short release latency (slot j's diag
                # only waits one neighbor's accum) at modest Scalar cost.
                groups = [(0, 2), (2, 2), (4, 2), (6, 1), (7, 1)] if last else [
                    (0, 2), (2, 2), (4, 2), (6, 2)
                ]
                for j0, glen in groups:
                    for j in range(j0, j0 + glen):
                        k = h * 8 + j
                        gc = gts[(hf, h)][j]
                        # bf16 product stream on DVE (2x rate); the fused
                        # tensor_tensor_reduce doesn't lower in this walrus
                        # build ("ISA wrong length"), so the free-axis
                        # accumulate alternates between Scalar (ACTIVATE
                        # accum_out) and DVE (tensor_reduce) so neither
                        # engine falls behind the 2.4us/slot gather drain.
                        scr = spool.tile([128, D], BF, tag="scr", bufs=3)
                        nc.vector.tensor_tensor(
                            out=scr[:],
                            in0=gc[:, 0:D],
                            in1=x_sb[hf][:],
                            op=mybir.AluOpType.mult,
                        )
                        # Scalar ACT accum is elem-paced (~1ns/elem bf16);
                        # DVE saturates at ~2.4us/slot if every slot folds.
                        # Alternate: odd slots fold on DVE (Scalar reads
                        # 1024), even slots skip it (Scalar reads 2048) -
                        # both engines land ~1.9-2.1us/slot, under the
                        # 2.38us/slot gather drain, so the consume eats the
                        # buffered backlog instead of riding it to the end.
                        fold = spool.tile(
                            [128, D // 2], BF, tag="fold", bufs=2
                        )
                        nc.vector.tensor_tensor(
                            out=fold[:],
                            in0=scr[:, 0 : D // 2],
                            in1=scr[:, D // 2 : D],
                            op=mybir.AluOpType.add,
                        )
                        scr2 = spool.tile([128, D // 2], BF, tag="scr2s", bufs=1)
                        nc.scalar.activation(
                            scr2[:],
                            fold[:],
                            mybir.ActivationFunctionType.Copy,
                            accum_out=hid_all[hf][:, k : k + 1],
                        )
                    k0 = h * 8 + j0
                    hsil4 = rpool.tile([128, 4], FP, tag="hsil", padded_shape=[128, 4])
                    nc.scalar.activation(
                        hsil4[:, 0:glen],
                        hid_all[hf][:, k0 : k0 + glen],
                        mybir.ActivationFunctionType.Silu,
                    )
                    nc.vector.tensor_tensor(
                        out=hs2_all[hf][:, k0 : k0 + glen],
                        in0=hsil4[:, 0:glen],
                        in1=fsr_all[hf][:, k0 : k0 + glen],
                        op=mybir.AluOpType.mult,
                    )
                    for j in range(j0, j0 + glen):
                        k = h * 8 + j
                        gc = gts[(hf, h)][j]
                        # diag build on Scalar (Copy with per-partition
                        # scale): with every slot folded, Scalar runs
                        # ~2.0us/slot and DVE ~1.9 - both under the 2.44
                        # drain so the consume backlog shrinks mid-stream.
                        dg = dgpool.tile([128, 128], BF, tag="dg")
                        nc.scalar.activation(
                            dg[:],
                            ident[:],
                            mybir.ActivationFunctionType.Copy,
                            scale=hs2_all[hf][:, k : k + 1],
                        )
                        for c4 in range(4):
                            nc.tensor.matmul(
                                acc[hf][:, c4 * 512 : (c4 + 1) * 512],
                                lhsT=dg[:],
                                rhs=gc[:, D + c4 * 512 : D + (c4 + 1) * 512],
                                start=(k == 0),
                                stop=(k == NSLOT - 1),
                            )
                if h == H - 1:
                    # chunked copy-out overlaps the tail matmuls and halves
                    # the exposed drain at the end of each half.
                    for c4 in range(4):
                        obc = opool.tile([128, 512], BF, tag="obc")
                        nc.scalar.activation(
                            obc[:],
                            acc[hf][:, c4 * 512 : (c4 + 1) * 512],
                            mybir.ActivationFunctionType.Copy,
                        )
                        nc.sync.dma_start(
                            out_d.ap()[t0 : t0 + 128, c4 * 512 : (c4 + 1) * 512],
                            obc[:],
                        )

            # Chains c=0..7 -> (hf, h) = (c // 4, c % 4).  Routing stays one
            # chain ahead of consumption; gathers are enqueued early and
            # self-pace against gather-buffer releases (nothing else runs on
            # gpsimd, so SWDGE stalls are harmless).  qt chunks sit in PE
            # idle gaps one full window before the routing that needs them.
            CH = [(0, 0), (0, 1), (0, 2), (0, 3), (1, 0), (1, 1), (1, 2), (1, 3)]
            # qt0b (chunk 0, tokens 128-255) only feeds half-1 routes, so
            # it runs AFTER qt1-3: each qt chunk is ~9us of serial fp32 PE
            # and having qt0b ahead of qt1 delayed routes 1-3 (and their
            # gathers) by that much.  ALL routes precede ALL consumes: the
            # 13-buffer gather stream covers the ~55us route phase, and the
            # mid/late-stream DVE queue is then pure consume work - a route
            # chain there stalled buffer releases; acc1 (PSUM pool shared
            # with the sims) also needs r7's sims done before chain 4.
            emit_qt(0, half=0)
            emit_route(*CH[0])
            emit_gather(*CH[0])
            emit_trailing_wq()
            emit_qt(1)
            emit_route(*CH[1])
            emit_gather(*CH[1])
            emit_qt(2)
            emit_route(*CH[2])
            emit_gather(*CH[2])
            emit_qt(3)
            emit_route(*CH[3])
            emit_gather(*CH[3])
            emit_qt(0, half=1)
            emit_route(*CH[4])
            emit_gather(*CH[4])
            emit_route(*CH[5])
            emit_gather(*CH[5])
            emit_route(*CH[6])
            emit_gather(*CH[6])
            emit_route(*CH[7])
            emit_gather(*CH[7])
            emit_consume(*CH[0])
            emit_consume(*CH[1])
            emit_consume(*CH[2])
            emit_consume(*CH[3])
            emit_consume(*CH[4])
            emit_consume(*CH[5])
            emit_consume(*CH[6])
            emit_consume(*CH[7])

    return nc


_CACHED = {}


def kernel(x, Wq, keys, e_down, e_up):
    x = np.asarray(x, dtype=np.float32)
    Wq = np.asarray(Wq, dtype=np.float32)
    keys = np.asarray(keys, dtype=np.float32)
    e_down = np.asarray(e_down, dtype=np.float32)
    e_up = np.asarray(e_up, dtype=np.float32)

    if "nc" not in _CACHED:
        _CACHED["nc"] = build_program()
    nc = _CACHED["nc"]

    xf = x.reshape(B * T, D)
    keyst = np.ascontiguousarray(keys.transpose(2, 3, 0, 1)).reshape(2 * DK, H * NK)
    # keyst[sub*64+dk, h*NK + nk] = keys[h, nk, sub, dk]
    identb = np.eye(128, dtype=ml_dtypes.bfloat16)
    iota64 = np.tile(np.arange(64, dtype=np.float32), (128, 1))
    NDCH = D // 128
    # wqs[p, m*16*128 + c*128 + o] = Wq[c*128+p, m*128+o] (m-major)
    wqs = np.ascontiguousarray(
        Wq.reshape(NDCH, 128, 4, 128).transpose(1, 2, 0, 3).reshape(128, NDCH * 512)
    )
    # one bf16 cat table: row e = [e_down[e] | e_up[e]]
    ecat = np.concatenate([e_down, e_up], axis=1).astype(ml_dtypes.bfloat16)

    in_maps = []
    for c in range(N_CORES):
        xs = np.ascontiguousarray(xf[c * TOK : (c + 1) * TOK])
        # xts[p, ch*TOK+t] = xs[t, ch*128+p]
        xts = np.ascontiguousarray(
            xs.reshape(TOK, NDCH, 128).transpose(2, 1, 0).reshape(128, NDCH * TOK)
        )
        in_maps.append(
            {
                "xb": xs.astype(ml_dtypes.bfloat16),
                "xts": xts,
                "wqs": wqs,
                "keyst": keyst,
                "ecat": ecat,
                "identb": identb,
                "iota64": iota64,
            }
        )

    res = run_bass_kernel_spmd(nc, in_maps, core_ids=list(range(N_CORES)))
    _CACHED["res"] = res
    out = np.concatenate(
        [res.results[c]["out"].astype(np.float32) for c in range(N_CORES)], axis=0
    )
    return out.reshape(B, T, D)
